# revision 123
# baseline (speedup 1.0000x reference)
"""AttnConv2d Trainium2 Bass kernel.

Reference computation (per image, batch B=16, C=64, H=W=96):
  kf = conv3x3(x1, w1); qf = conv3x3(x2, w2); vf = conv3x3(x1, w3)
  key/qry = stride-3 non-overlapping 3x3 patch unfold of kf/qf
  scores[k, c, d] = sum_l key[k][c, l] * qry[k][d, l]   (k = patch class 0..8)
  attn = softmax(scores^T flattened / 24)
  out[d, x] = sum_{c,t} attn[d, c, t] * vf_pad[c, x + off(t)]

Sharding: pure data parallel, 2 images per NeuronCore across 8 cores.

v3 design notes (cost model: matmul time = out free size N x 0.417ns,
independent of M and K; PE decode ~2ns/instr; pstate ramps over 3us):
 - every matmul is a single 128-wide instruction:
   * kf|vf conv merged in the M dim (both consume x1): lhsT [128,128] with
     kf weights in one column half, vf in the other; K=128 carries 2 conv
     taps via the 2-copy row-shifted x1 layout; image 1 runs 5 matmuls per
     3-row strip using a T2 = [x+2HP | x+2HP+1] tile (built by two cheap
     same-partition shift copies of T1) that pairs the (2,0)/(2,1) taps.
   * qf conv image-merged in K: rhs tile holds [x2_img0 | x2_img1] in the
     two partition halves, block-diagonal lhsT (the M=64 floor: the ISA
     limits the stationary operand to ONE free dim, so a transposed
     pixel-major-M conv is not expressible). 9 matmuls/strip, BOTH images.
   * output einsum image-merged the same way over [vf_i1 | vf_i0].
 - weight lhsT tiles and the 128x128 identity are packed on the host and
   DMA'd in (startup single-tap slots first -- they gate strip 0).
 - kf/qf conv outputs stored patch-class-grouped chunk-major; batched DMA
   transposes ([64, 2-4 chunks x 1152] -> [128, ., 9, 64]) keep the HWDGE
   instruction count low; the image-1 kf transposes run at 2-chunk grain so
   scores wait only ~1us after the last conv strip.
 - scores accumulate 8+1 classes side by side in two PSUM banks
   (class-major: interleaved open accumulation groups in one bank
   MISACCUMULATE on hardware) and leave through the softmax exp directly;
   image-0's whole scores/softmax pipeline overlaps image-1 work.
 - attnT holds UNNORMALIZED exp in bf16; per-row 1/sum factors (rs128)
   fold into the output-copy activation scale.
 - PSUM: conv pool 4 banks + scores/output pool 4 banks; attn transposes
   borrow idle conv-pool banks.
"""

import numpy as np

try:
    import concourse.bass as bass  # noqa: F401
except Exception:  # pragma: no cover - path fallback for fresh containers
    import sys

    for p in ("/opt/trn_rl_repo", "/root/.axon_site/_ro/trn_rl_repo"):
        if p not in sys.path:
            sys.path.append(p)
    import concourse.bass as bass  # noqa: F401

from contextlib import ExitStack

import concourse.mybir as mybir
import concourse.tile as tile
from concourse import bacc
from concourse.bass_utils import run_bass_kernel_spmd

F32 = mybir.dt.float32
BF16 = mybir.dt.bfloat16

B, C, H, W = 16, 64, 96, 96
NCORES = 8
BPC = B // NCORES  # images per core
HP = H + 2  # padded row length
NPAD = HP * HP
NCHK = 4  # input load staged in quarter-image chunks
NSTRIP = H // 3  # 32 grouped strips (3 rows = one patch-row)
VSTRIP = H // 4  # 24 spatial strips (4 rows, N=384) for the output einsum
AA = 9  # patch classes / taps
L = (H // 3) * (W // 3)  # 1024 patches
SCALE = 1.0 / 24.0  # 1/sqrt(64*9)
# Wkv slot map: 0-2 row pairs, 3 column pair (2,0)&(2,1), 4-6 singles
# (2,2)/(2,0)/(2,1), 7-12 singles (0,0)..(1,2) for the startup strips
SINGLE_SLOTS = [
    (7, (0, 0)), (8, (0, 1)), (9, (0, 2)),
    (10, (1, 0)), (11, (1, 1)), (12, (1, 2)),
    (5, (2, 0)), (6, (2, 1)), (4, (2, 2)),
]
EXP_BIAS = -12.0  # constant softmax shift; |scores|/24 stays well below 12

_CACHE = {}
_DEBUG = False
_IDN = np.eye(128, dtype=np.float32)
_PHASES = []  # (phase_name, first_instruction_number) build-time markers


def _build_program():
    nc = bacc.Bacc("TRN2", target_bir_lowering=False, debug=False)

    x1c = nc.dram_tensor("x1c", [BPC, C, H, W], F32, kind="ExternalInput")
    x2c = nc.dram_tensor("x2c", [BPC, C, H, W], F32, kind="ExternalInput")
    w1 = nc.dram_tensor("w1", [C, C, 3, 3], F32, kind="ExternalInput")
    w2 = nc.dram_tensor("w2", [C, C, 3, 3], F32, kind="ExternalInput")
    w3 = nc.dram_tensor("w3", [C, C, 3, 3], F32, kind="ExternalInput")
    yc = nc.dram_tensor("yc", [BPC, C, H, W], F32, kind="ExternalOutput")
    idn = nc.dram_tensor("idn", [128, 128], F32, kind="ExternalInput")
    wkv0d = nc.dram_tensor("wkv0", [128, 13 * 128], BF16, kind="ExternalInput")
    wkv1d = nc.dram_tensor("wkv1", [128, 13 * 128], BF16, kind="ExternalInput")
    wqd = nc.dram_tensor("wq", [128, 9 * 128], BF16, kind="ExternalInput")

    def mark(phase):
        _PHASES.append((phase, int(nc.get_next_instruction_name().split("-")[1])))

    with ExitStack() as ctx:
        tc = ctx.enter_context(tile.TileContext(nc))

        wpool = ctx.enter_context(tc.tile_pool(name="wpool", bufs=1))
        xpool = ctx.enter_context(tc.tile_pool(name="xpool", bufs=1))
        stpool = ctx.enter_context(tc.tile_pool(name="stpool", bufs=3))
        cvpool = ctx.enter_context(tc.tile_pool(name="cvpool", bufs=1))
        tpool = ctx.enter_context(tc.tile_pool(name="tpool", bufs=1))
        obpool = ctx.enter_context(tc.tile_pool(name="obpool", bufs=2))

        psC = ctx.enter_context(tc.tile_pool(name="psC", bufs=4, space="PSUM"))
        psO = ctx.enter_context(tc.tile_pool(name="psO", bufs=4, space="PSUM"))

        def conv_psum():
            return psC.tile([128, 288], F32, name="ps_conv", tag="conv")

        mark('weights')
        # lhsT weight tiles and the identity arrive pre-packed from the host
        # (pure layout transforms of w1/w2/w3 -- no FLOPs): this removes the
        # on-device transpose machinery from the startup critical path.
        identf128 = wpool.tile([128, 128], F32, name="identf128")
        Wkv0 = wpool.tile([128, 13, 128], BF16, name="Wkv0")
        Wkv1 = wpool.tile([128, 13, 128], BF16, name="Wkv1")
        Wq = wpool.tile([128, 9, 128], BF16, name="Wq")
        wkv0f = Wkv0.rearrange("p a b -> p (a b)")
        # single-tap slots (4-12) land first: they gate the startup strips
        nc.sync.dma_start(
            out=wkv0f[:, 512:1664], in_=wkv0d.rearrange("a b -> a b")[:, 512:1664]
        )
        nc.scalar.dma_start(
            out=wkv0f[:, 0:512], in_=wkv0d.rearrange("a b -> a b")[:, 0:512]
        )
        nc.scalar.dma_start(
            out=Wq.rearrange("p a b -> p (a b)"), in_=wqd.rearrange("a b -> a b")
        )
        nc.scalar.dma_start(out=identf128, in_=idn.rearrange("a b -> a b"))
        nc.scalar.dma_start(
            out=Wkv1.rearrange("p a b -> p (a b)"), in_=wkv1d.rearrange("a b -> a b")
        )
        identf64 = identf128[0:64, 0:64]
        identb64 = wpool.tile([64, 64], BF16, name="identb64")
        nc.scalar.copy(out=identb64, in_=identf64)

        # ------------------------------------------------------------------
        # conv output tiles
        # ------------------------------------------------------------------
        # K_f: patch-class-grouped kf, [kf_i0 | kf_i1] in partition halves.
        # K_q: same for qf. T_v: padded vf, [vf_i1 | vf_i0].
        K_f = cvpool.tile([128, AA * L], BF16, name="K_f")
        K_q = cvpool.tile([128, AA * L], BF16, name="K_q")
        T_v = cvpool.tile([128, NPAD], BF16, name="T_v")
        tvv = T_v.rearrange("p (h w) -> p h w", h=HP, w=HP)

        def tv_borders():
            nc.vector.memset(tvv[:, 0:1, :], 0.0)
            nc.vector.memset(tvv[:, HP - 1 : HP, :], 0.0)
            nc.vector.memset(tvv[:, 1 : HP - 1, 0:1], 0.0)
            nc.vector.memset(tvv[:, 1 : HP - 1, HP - 1 : HP], 0.0)

        # ------------------------------------------------------------------
        # input staging
        # ------------------------------------------------------------------
        N1 = 8  # x1 load chunks (12 rows each): fine-grained conv feed
        rows1 = H // N1
        CHB1 = rows1 * W
        CB1 = rows1 * HP
        rows = H // NCHK
        CHB = rows * W  # chunk elements per channel

        def xt_tile(name):
            # T1_0 / T_q / T1_1 / T2_1 rotate through three buffers: T2_1
            # lands in T1_0's slot once kfvf_i0 has consumed it.
            return xpool.tile([128, NPAD], BF16, name=name, tag="xT", bufs=3)

        def t1_alloc(name):
            """[x1_i | x1_i + HP] 2-copy padded tile, borders pre-zeroed."""
            xp = xt_tile(name)
            v = xp.rearrange("p (h w) -> p h w", h=HP, w=HP)
            nc.vector.memset(v[0:64, 0:1, :], 0.0)
            nc.vector.memset(v[0:64, HP - 1 : HP, :], 0.0)
            nc.vector.memset(v[0:64, 1 : HP - 1, 0:1], 0.0)
            nc.vector.memset(v[0:64, 1 : HP - 1, HP - 1 : HP], 0.0)
            nc.vector.memset(v[64:128, 0 : HP - 1, 0:1], 0.0)
            nc.vector.memset(v[64:128, 0 : HP - 1, HP - 1 : HP], 0.0)
            nc.vector.memset(xp[64:128, (HP - 2) * HP : NPAD], 0.0)
            return xp

        def t1_rows(xp, img, a, b, lower_q=None):
            v = xp.rearrange("p (h w) -> p h w", h=HP, w=HP)
            st = stpool.tile([64, CHB1], BF16, name="st", tag="st", bufs=5)
            stc = st[:, 0 : (b - a) * W]
            nc.gpsimd.dma_start(
                out=stc, in_=x1c[img][:, a:b].rearrange("c h w -> c (h w)")
            )
            stv = stc.rearrange("p (h w) -> p h w", h=b - a, w=W)
            nc.vector.tensor_copy(out=v[0:64, 1 + a : 1 + b, 1 : W + 1], in_=stv)
            (lower_q or nc.sync).dma_start(
                out=v[64:128, a:b, 1 : W + 1], in_=stv
            )

        def t1_chunk(xp, img, g):
            t1_rows(xp, img, g * rows1, (g + 1) * rows1)

        def t2_quarter(t2, T1, g):
            """T2 = [x+2HP | x+2HP+1] via same-partition shift copies: the
            upper half shifts T1's upper by 2 rows, the lower half shifts
            T1's lower (x+HP) by HP+1."""
            e = 2352 * (g + 1)
            nc.sync.dma_start(
                out=t2[0:64, 2352 * g : e], in_=T1[0:64, 196 + 2352 * g : 196 + e]
            )
            nc.sync.dma_start(
                out=t2[64:128, 2352 * g : e], in_=T1[64:128, 99 + 2352 * g : 99 + e]
            )
            if g == 3:
                nc.vector.memset(t2[0:64, 9408:NPAD], 0.0)
                nc.vector.memset(t2[64:128, 9408:NPAD], 0.0)

        def build_Tq():
            """[x2_i0 | x2_i1] padded tile, loaded straight from DRAM with
            strided cast-DMAs in row quarters (nothing on the DVE queue)."""
            xp = xt_tile("T_q")
            v = xp.rearrange("p (h w) -> p h w", h=HP, w=HP)
            nc.vector.memset(v[:, 0:1, :], 0.0)
            nc.vector.memset(v[:, HP - 1 : HP, :], 0.0)
            nc.vector.memset(v[:, 1 : HP - 1, 0:1], 0.0)
            nc.vector.memset(v[:, 1 : HP - 1, HP - 1 : HP], 0.0)
            xcv0 = x2c[0].rearrange("c (g h) w -> g c h w", g=4)
            xcv1 = x2c[1].rearrange("c (g h) w -> g c h w", g=4)
            for g in range(4):
                r0 = 1 + g * 24
                nc.gpsimd.dma_start(
                    out=v[0:64, r0 : r0 + 24, 1 : W + 1], in_=xcv0[g]
                )
                nc.gpsimd.dma_start(
                    out=v[64:128, r0 : r0 + 24, 1 : W + 1], in_=xcv1[g]
                )
            return xp

        mark('loads')
        T1_0 = t1_alloc("T1_0")
        # the very first rows land as a half-chunk so strip 0 starts sooner
        t1_rows(T1_0, 0, 0, 6)
        t1_rows(T1_0, 0, 6, 12)
        for g in range(1, N1):
            t1_chunk(T1_0, 0, g)

        # ------------------------------------------------------------------
        # convolutions (3-row grouped strips, N=288)
        # ------------------------------------------------------------------
        # chunk-major grouped layout: chunk s (128 patches) of all 9 classes
        # is contiguous, so one DMA transpose per chunk can fire as soon as
        # its 4 source strips are done (overlapping the conv itself).
        kfA = K_f.rearrange("p (s ki kj l) -> p s ki kj l", s=8, ki=3, kj=3)
        kqA = K_q.rearrange("p (s ki kj l) -> p s ki kj l", s=8, ki=3, kj=3)

        def gr(xv, h0, hs, dj):
            return xv[:, h0 + hs : h0 + hs + 3, dj : dj + W].rearrange(
                "p ki (b kj) -> p ki kj b", kj=3
            )

        def kfvf_strip(img, T1, T2, Wkv, r, use_t2, singles_only=False):
            x1v = T1.rearrange("p (h w) -> p h w", h=HP, w=HP)
            h0 = 3 * r
            ps = conv_psum()
            pv = ps.rearrange("p (a b c) -> p a b c", a=3, b=3)
            grl = lambda hs, dj: gr(x1v[0:64], h0, hs, dj)
            if singles_only:
                # 9 K=64 single-tap matmuls touching only T1's upper half --
                # lets the first strips run before the row-shifted copy lands
                n = 0
                for slot, (hs, dj) in SINGLE_SLOTS:
                    nc.tensor.matmul(
                        pv,
                        Wkv[0:64, slot, :],
                        grl(hs, dj),
                        start=(n == 0),
                        stop=(n == 8),
                    )
                    n += 1
            else:
                for j in range(3):
                    nc.tensor.matmul(
                        pv, Wkv[:, j, :], gr(x1v, h0, 0, j), start=(j == 0), stop=False
                    )
                if use_t2:
                    # column-pair (2,0)&(2,1) via the [x+2HP | x+2HP+1] tile,
                    # then the lone (2,2) single
                    t2v = T2.rearrange("p (h w) -> p h w", h=HP, w=HP)
                    nc.tensor.matmul(
                        pv, Wkv[:, 3, :], gr(t2v, h0, 0, 0), start=False, stop=False
                    )
                    nc.tensor.matmul(
                        pv, Wkv[0:64, 4, :], grl(2, 2), start=False, stop=True
                    )
                else:
                    for n, (slot, dj) in enumerate(((5, 0), (6, 1), (4, 2))):
                        nc.tensor.matmul(
                            pv,
                            Wkv[0:64, slot, :],
                            grl(2, dj),
                            start=False,
                            stop=(n == 2),
                        )
            pk = ps.rearrange("p (ki kj c) -> p ki kj c", ki=3, kj=3)
            kdst = kfA[:, r // 4, :, :, 32 * (r % 4) : 32 * (r % 4) + 32]
            vdst = tvv[:, 1 + h0 : 4 + h0, 1 : W + 1].rearrange(
                "p ki (b kj) -> p ki kj b", kj=3
            )
            if img == 0:
                # psum = [kf_i0 | vf_i0]
                nc.scalar.copy(out=kdst[0:64], in_=pk[0:64])
                nc.vector.tensor_copy(out=vdst[64:128], in_=pk[64:128])
            else:
                # psum = [vf_i1 | kf_i1]
                nc.vector.tensor_copy(out=vdst[0:64], in_=pk[0:64])
                nc.scalar.copy(out=kdst[64:128], in_=pk[64:128])

        def qf_strip(r):
            xqv = T_q.rearrange("p (h w) -> p h w", h=HP, w=HP)
            h0 = 3 * r
            ps = conv_psum()
            pv = ps.rearrange("p (a b c) -> p a b c", a=3, b=3)
            t = 0
            for hs in range(3):
                for dj in range(3):
                    nc.tensor.matmul(
                        pv,
                        Wq[:, t, :],
                        gr(xqv, h0, hs, dj),
                        start=(t == 0),
                        stop=(t == 8),
                    )
                    t += 1
            pk = ps.rearrange("p (ki kj c) -> p ki kj c", ki=3, kj=3)
            qdst = kqA[:, r // 4, :, :, 32 * (r % 4) : 32 * (r % 4) + 32]
            nc.scalar.copy(out=qdst, in_=pk)

        # transposes: [64, (4 chunks)*9*128] -> [128, 4, 9, 64] per image
        # half, batched 2 per image to keep the HWDGE instruction count low
        # (out[p, s, k, c] = chunk s, class k, local patch p)
        kTall = [
            tpool.tile([128, 8, AA, 64], BF16, name=f"kTall{i}") for i in range(2)
        ]
        qTall = [
            tpool.tile([128, 8, AA, 64], BF16, name=f"qTall{i}") for i in range(2)
        ]

        def half_transpose(dst, img, K_tile, h):
            half = slice(0, 64) if img == 0 else slice(64, 128)
            nc.sync.dma_start_transpose(
                dst[img][:, 4 * h : 4 * h + 4],
                K_tile[half, 4608 * h : 4608 * (h + 1)],
            )

        def half_transpose2(dst, img, K_tile, h2):
            half = slice(0, 64) if img == 0 else slice(64, 128)
            nc.sync.dma_start_transpose(
                dst[img][:, 2 * h2 : 2 * h2 + 2],
                K_tile[half, 2304 * h2 : 2304 * (h2 + 1)],
            )

        def half_transpose1(dst, img, K_tile, h1):
            half = slice(0, 64) if img == 0 else slice(64, 128)
            nc.sync.dma_start_transpose(
                dst[img][:, h1 : h1 + 1],
                K_tile[half, 1152 * h1 : 1152 * (h1 + 1)],
            )

        # schedule: kfvf i0; qf in the middle (its 35us window hides image-1
        # input staging and the T2_1 build, and its 1080ns strips leave evac
        # slack); kfvf i1 with the pair-tap T2 last.
        mark('kfvf_i0')
        for r in range(NSTRIP):
            kfvf_strip(0, T1_0, None, Wkv0, r, use_t2=False, singles_only=(r < 1))
            if r % 16 == 15:
                half_transpose(kTall, 0, K_f, r // 16)
        tv_borders()
        T_q = build_Tq()
        T1_1 = t1_alloc("T1_1")
        T2_1 = xt_tile("T2_1")
        _T2_AFTER = {2: 0, 4: 1, 6: 2, 7: 3}
        for g in range(N1):
            t1_chunk(T1_1, 1, g)
            if g in _T2_AFTER:
                t2_quarter(T2_1, T1_1, _T2_AFTER[g])
        mark('qf')
        for r in range(NSTRIP):
            qf_strip(r)
            if r % 16 == 15:
                half_transpose(qTall, 0, K_q, r // 16)
                half_transpose(qTall, 1, K_q, r // 16)
        mark('kfvf_i1')
        for r in range(NSTRIP):
            kfvf_strip(1, T1_1, T2_1, Wkv1, r, use_t2=True)
            if r in (7, 15):
                half_transpose2(kTall, 1, K_f, r // 8)
            elif r in (19, 23, 27, 31):
                half_transpose1(kTall, 1, K_f, (r - 19) // 4 + 4)

        # ------------------------------------------------------------------
        # scores + softmax + attn kernel transposes
        # ------------------------------------------------------------------
        # sc rows = d (partitions 0-63), both images side by side in free dim.
        mark('scores')
        attnP = [
            cvpool.tile([64, AA, 64], BF16, name=f"attnP{i}") for i in range(2)
        ]
        ebias = cvpool.tile([64, 1], F32, name="ebias")
        nc.vector.memset(ebias, EXP_BIAS)
        # attnT[:, t, :]: K rows 0-63 = c of i1 -> M cols 0-63 = d of i1;
        # K rows 64-127 = c of i0 -> M cols 64-127 = d of i0; off-diag zero.
        attnT = cvpool.tile([128, AA, 128], BF16, name="attnT")
        nc.vector.memset(attnT, 0.0)

        def scores_img(img, k0, k1, ps):
            # classes k0..k1-1 accumulate side by side in one PSUM bank and
            # leave through the softmax exp directly (no sc staging tile).
            # class-major: exactly one open accumulation group per bank at a
            # time (interleaved open groups misaccumulate on hardware).
            for k in range(k0, k1):
                po = ps[0:64, 64 * (k - k0) : 64 * (k - k0) + 64]
                for s in range(8):
                    nc.tensor.matmul(
                        po,
                        qTall[img][:, s, k, :],
                        kTall[img][:, s, k, :],
                        start=(s == 0),
                        stop=(s == 7),
                    )

        # attnT holds UNNORMALIZED exp values; the per-row 1/sum factors are
        # gathered into rs128 (rows 0-63 = img1, 64-127 = img0, matching the
        # output einsum psum rows) and applied by the output-copy activation.
        rs128 = cvpool.tile([128, 1], F32, name="rs128")

        sm_acc = [
            [cvpool.tile([64, 1], F32, name=f"sm{i}{j}") for j in range(3)]
            for i in range(2)
        ]
        _EXP_RNG = [slice(0, 256), slice(256, 512), slice(512, 576)]

        def exp_part(img, ps, part):
            """exp straight out of one scores PSUM sub-range; fires as soon
            as those classes' accumulation stops (others may still run)."""
            exf = attnP[img].rearrange("p a c -> p (a c)")
            nc.scalar.activation(
                out=exf[:, _EXP_RNG[part]],
                in_=ps,
                func=mybir.ActivationFunctionType.Exp,
                bias=ebias,
                scale=SCALE,
                accum_out=sm_acc[img][part],
            )

        def finish_softmax(img):
            smp = cvpool.tile([64, 1], F32, name=f"smp{img}")
            nc.scalar.add(out=smp, in_=sm_acc[img][0], add=sm_acc[img][1])
            sm = cvpool.tile([64, 1], F32, name=f"sm{img}")
            nc.scalar.add(out=sm, in_=smp, add=sm_acc[img][2])
            rs = cvpool.tile([64, 1], F32, name=f"rs{img}")
            nc.vector.reciprocal(rs, sm)
            # cross-partition move via DMA; latency is hidden (needed only
            # by the first output copy, ~15us later)
            nc.sync.dma_start(
                out=rs128[64:128, :] if img == 0 else rs128[0:64, :], in_=rs
            )

        # pipeline: img0 scores+softmax+attnT-half run while img1's kT
        # transposes land; only img1's chain sits on the critical path.
        def attn_t_batch(half, t0, nt=3):
            # attnT built in 3-tap batches out of the (now idle) conv psum
            # pool -- 4-deep rotation so batches never wait on each other
            ps = psC.tile([128, 64 * nt], F32, name="ps_t", tag="conv")
            pv = ps.rearrange("p (t c) -> p t c", t=nt)
            for t in range(t0, t0 + nt):
                o = 64 * (t - t0)
                if half == 0:
                    nc.tensor.matmul(
                        ps[64:128, o : o + 64], attnP[0][:, t, :], identb64
                    )
                else:
                    nc.tensor.matmul(
                        ps[0:64, o : o + 64], attnP[1][:, t, :], identb64
                    )
            if half == 0:
                nc.vector.tensor_copy(
                    out=attnT[64:128, t0 : t0 + nt, 64:128],
                    in_=pv[64:128, 0:nt, :],
                )
            else:
                nc.vector.tensor_copy(
                    out=attnT[0:64, t0 : t0 + nt, 0:64],
                    in_=pv[0:64, 0:nt, :],
                )

        psA0 = psO.tile([64, 512], F32, name="ps_sA", tag="po")
        psB0 = psO.tile([64, 64], F32, name="ps_sB", tag="po")
        scores_img(0, 0, 4, psA0[:, 0:256])
        exp_part(0, psA0[:, 0:256], 0)
        scores_img(0, 4, 8, psA0[:, 256:512])
        exp_part(0, psA0[:, 256:512], 1)
        scores_img(0, 8, 9, psB0)
        exp_part(0, psB0, 2)
        finish_softmax(0)
        psA1 = psO.tile([64, 512], F32, name="ps_sA", tag="po")
        psB1 = psO.tile([64, 64], F32, name="ps_sB", tag="po")
        attn_t_batch(0, 0)
        scores_img(1, 0, 4, psA1[:, 0:256])
        exp_part(1, psA1[:, 0:256], 0)
        attn_t_batch(0, 3)
        scores_img(1, 4, 8, psA1[:, 256:512])
        exp_part(1, psA1[:, 256:512], 1)
        attn_t_batch(0, 6)
        scores_img(1, 8, 9, psB1)
        exp_part(1, psB1, 2)
        finish_softmax(1)
        mark('softmax')
        attn_t_batch(1, 0)
        attn_t_batch(1, 3)
        attn_t_batch(1, 6)

        mark('cein')
        y0 = yc[0].rearrange("c h w -> c (h w)")
        y1 = yc[1].rearrange("c h w -> c (h w)")
        def c_strip(h0, nrow, last=False):
            npix = nrow * W
            ps = psO.tile([128, 384], F32, name="ps_o", tag="po")
            pov = ps[:, 0:npix].rearrange("p (a c) -> p a c", a=nrow)
            t = 0
            for ti in range(3):
                for tj in range(3):
                    nc.tensor.matmul(
                        pov,
                        attnT[:, t, :],
                        tvv[:, h0 + ti : h0 + ti + nrow, tj : tj + W],
                        start=(t == 0),
                        stop=(t == 8),
                    )
                    t += 1
            ob = obpool.tile([128, 384], F32, name="outb", tag="outb", bufs=3)
            nc.scalar.activation(
                out=ob[:, 0:npix],
                in_=ps[:, 0:npix],
                func=mybir.ActivationFunctionType.Copy,
                scale=rs128,
            )
            # y1 transfers alternate sync/scalar so neither sequencer's
            # per-DMA hold backlogs the final strip
            q1 = nc.sync if (h0 // 4) % 2 == 0 else nc.scalar
            q1.dma_start(
                out=y1[:, W * h0 : W * (h0 + nrow)], in_=ob[0:64, 0:npix]
            )
            # image-0's transfers ride the idle Pool queue throughout so the
            # sync queue only carries part of the output DMAs
            nc.gpsimd.dma_start(
                out=y0[:, W * h0 : W * (h0 + nrow)], in_=ob[64:128, 0:npix]
            )

        for r in range(VSTRIP):
            c_strip(4 * r, 4, last=(r == VSTRIP - 1))

        if _DEBUG:
            dbg = {
                "dK_f": K_f, "dK_q": K_q, "dT_v": T_v,
                "dkT0": kTall[0].rearrange("p a k c -> p (a k c)"),
                "dqT0": qTall[0].rearrange("p a k c -> p (a k c)"),
                "daP0": attnP[0].rearrange("p a c -> p (a c)"),
                "daP1": attnP[1].rearrange("p a c -> p (a c)"),
                "datT": attnT.rearrange("p a c -> p (a c)"),
                "drs": rs128,
            }
            for nm, t in dbg.items():
                sh = [t.shape[0], int(np.prod(t.shape[1:]))]
                dt_ = F32 if nm == "drs" else (BF16 if t.dtype == BF16 else F32)
                d = nc.dram_tensor(nm, sh, t.dtype, kind="ExternalOutput")
                nc.sync.dma_start(out=d.rearrange("a b -> a b"), in_=t)

    mark('end')
    nc.compile()
    return nc


def _get_program():
    if "nc" not in _CACHE:
        _CACHE["nc"] = _build_program()
    return _CACHE["nc"]


def _pack_kv(wa, wb):
    """lhsT tile for the merged kf|vf conv: [K=128, slot, M=128] flattened."""
    out = np.zeros((128, 13, 128), np.float32)
    for j in range(3):
        for a in range(2):
            out[a * 64 : (a + 1) * 64, j, 0:64] = wa[:, :, a, j].T
            out[a * 64 : (a + 1) * 64, j, 64:128] = wb[:, :, a, j].T
    for b in range(2):
        out[b * 64 : (b + 1) * 64, 3, 0:64] = wa[:, :, 2, b].T
        out[b * 64 : (b + 1) * 64, 3, 64:128] = wb[:, :, 2, b].T
    for slot, dj in ((4, 2), (5, 0), (6, 1)):
        out[0:64, slot, 0:64] = wa[:, :, 2, dj].T
        out[0:64, slot, 64:128] = wb[:, :, 2, dj].T
    slot = 7
    for a in range(2):
        for dj in range(3):
            out[0:64, slot, 0:64] = wa[:, :, a, dj].T
            out[0:64, slot, 64:128] = wb[:, :, a, dj].T
            slot += 1
    import ml_dtypes

    return out.reshape(128, 13 * 128).astype(ml_dtypes.bfloat16)


def _pack_q(w2):
    """Block-diagonal image-merged qf lhsT tiles."""
    out = np.zeros((128, 9, 128), np.float32)
    t = 0
    for ti in range(3):
        for tj in range(3):
            blk = w2[:, :, ti, tj].T
            out[0:64, t, 0:64] = blk
            out[64:128, t, 64:128] = blk
            t += 1
    import ml_dtypes

    return out.reshape(128, 9 * 128).astype(ml_dtypes.bfloat16)


def kernel(x1, x2, w1, w2, w3, **kwargs):
    x1 = np.ascontiguousarray(np.asarray(x1, dtype=np.float32))
    x2 = np.ascontiguousarray(np.asarray(x2, dtype=np.float32))
    w1 = np.ascontiguousarray(np.asarray(w1, dtype=np.float32))
    w2 = np.ascontiguousarray(np.asarray(w2, dtype=np.float32))
    w3 = np.ascontiguousarray(np.asarray(w3, dtype=np.float32))
    wkv0 = _pack_kv(w1, w3)
    wkv1 = _pack_kv(w3, w1)
    wq = _pack_q(w2)

    nc = _get_program()
    in_maps = [
        {
            "x1c": x1[i * BPC : (i + 1) * BPC],
            "x2c": x2[i * BPC : (i + 1) * BPC],
            "w1": w1,
            "w2": w2,
            "w3": w3,
            "idn": _IDN,
            "wkv0": wkv0,
            "wkv1": wkv1,
            "wq": wq,
        }
        for i in range(NCORES)
    ]
    try:
        res = run_bass_kernel_spmd(
            nc, in_maps, core_ids=list(range(NCORES)), **kwargs
        )
    except Exception:
        # one retry: transient device state can fail a first attempt
        res = run_bass_kernel_spmd(
            nc, in_maps, core_ids=list(range(NCORES)), **kwargs
        )
    out = np.concatenate([r["yc"] for r in res.results], axis=0)
    if kwargs:
        return out.astype(np.float32), res
    return out.astype(np.float32)



# revision 126
# speedup vs baseline: 1.0022x; 1.0022x over previous
"""AttnConv2d Trainium2 Bass kernel.

Reference computation (per image, batch B=16, C=64, H=W=96):
  kf = conv3x3(x1, w1); qf = conv3x3(x2, w2); vf = conv3x3(x1, w3)
  key/qry = stride-3 non-overlapping 3x3 patch unfold of kf/qf
  scores[k, c, d] = sum_l key[k][c, l] * qry[k][d, l]   (k = patch class 0..8)
  attn = softmax(scores^T flattened / 24)
  out[d, x] = sum_{c,t} attn[d, c, t] * vf_pad[c, x + off(t)]

Sharding: pure data parallel, 2 images per NeuronCore across 8 cores.

v3 design notes (cost model: matmul time = out free size N x 0.417ns,
independent of M and K; PE decode ~2ns/instr; pstate ramps over 3us):
 - every matmul is a single 128-wide instruction:
   * kf|vf conv merged in the M dim (both consume x1): lhsT [128,128] with
     kf weights in one column half, vf in the other; K=128 carries 2 conv
     taps via the 2-copy row-shifted x1 layout; image 1 runs 5 matmuls per
     3-row strip using a T2 = [x+2HP | x+2HP+1] tile (built by two cheap
     same-partition shift copies of T1) that pairs the (2,0)/(2,1) taps.
   * qf conv image-merged in K: rhs tile holds [x2_img0 | x2_img1] in the
     two partition halves, block-diagonal lhsT (the M=64 floor: the ISA
     limits the stationary operand to ONE free dim, so a transposed
     pixel-major-M conv is not expressible). 9 matmuls/strip, BOTH images.
   * output einsum image-merged the same way over [vf_i1 | vf_i0].
 - weight lhsT tiles and the 128x128 identity are packed on the host and
   DMA'd in (startup single-tap slots first -- they gate strip 0).
 - kf/qf conv outputs stored patch-class-grouped chunk-major; batched DMA
   transposes ([64, 2-4 chunks x 1152] -> [128, ., 9, 64]) keep the HWDGE
   instruction count low; the image-1 kf transposes run at 2-chunk grain so
   scores wait only ~1us after the last conv strip.
 - scores accumulate 8+1 classes side by side in two PSUM banks
   (class-major: interleaved open accumulation groups in one bank
   MISACCUMULATE on hardware) and leave through the softmax exp directly;
   image-0's whole scores/softmax pipeline overlaps image-1 work.
 - attnT holds UNNORMALIZED exp in bf16; per-row 1/sum factors (rs128)
   fold into the output-copy activation scale.
 - PSUM: conv pool 4 banks + scores/output pool 4 banks; attn transposes
   borrow idle conv-pool banks.
"""

import numpy as np

try:
    import concourse.bass as bass  # noqa: F401
except Exception:  # pragma: no cover - path fallback for fresh containers
    import sys

    for p in ("/opt/trn_rl_repo", "/root/.axon_site/_ro/trn_rl_repo"):
        if p not in sys.path:
            sys.path.append(p)
    import concourse.bass as bass  # noqa: F401

from contextlib import ExitStack

import concourse.mybir as mybir
import concourse.tile as tile
from concourse import bacc
from concourse.bass_utils import run_bass_kernel_spmd

F32 = mybir.dt.float32
BF16 = mybir.dt.bfloat16

B, C, H, W = 16, 64, 96, 96
NCORES = 8
BPC = B // NCORES  # images per core
HP = H + 2  # padded row length
NPAD = HP * HP
NCHK = 4  # input load staged in quarter-image chunks
NSTRIP = H // 3  # 32 grouped strips (3 rows = one patch-row)
VSTRIP = H // 4  # 24 spatial strips (4 rows, N=384) for the output einsum
AA = 9  # patch classes / taps
L = (H // 3) * (W // 3)  # 1024 patches
SCALE = 1.0 / 24.0  # 1/sqrt(64*9)
# Wkv slot map: 0-2 row pairs, 3 column pair (2,0)&(2,1), 4-6 singles
# (2,2)/(2,0)/(2,1), 7-12 singles (0,0)..(1,2) for the startup strips
SINGLE_SLOTS = [
    (7, (0, 0)), (8, (0, 1)), (9, (0, 2)),
    (10, (1, 0)), (11, (1, 1)), (12, (1, 2)),
    (5, (2, 0)), (6, (2, 1)), (4, (2, 2)),
]
EXP_BIAS = -12.0  # constant softmax shift; |scores|/24 stays well below 12

_CACHE = {}
_DEBUG = False
_IDN = np.eye(128, dtype=np.float32)
_PHASES = []  # (phase_name, first_instruction_number) build-time markers


def _build_program():
    nc = bacc.Bacc("TRN2", target_bir_lowering=False, debug=False)

    x1c = nc.dram_tensor("x1c", [BPC, C, H, W], F32, kind="ExternalInput")
    x2c = nc.dram_tensor("x2c", [BPC, C, H, W], F32, kind="ExternalInput")
    w1 = nc.dram_tensor("w1", [C, C, 3, 3], F32, kind="ExternalInput")
    w2 = nc.dram_tensor("w2", [C, C, 3, 3], F32, kind="ExternalInput")
    w3 = nc.dram_tensor("w3", [C, C, 3, 3], F32, kind="ExternalInput")
    yc = nc.dram_tensor("yc", [BPC, C, H, W], F32, kind="ExternalOutput")
    idn = nc.dram_tensor("idn", [128, 128], F32, kind="ExternalInput")
    wkv0d = nc.dram_tensor("wkv0", [128, 13 * 128], BF16, kind="ExternalInput")
    wkv1d = nc.dram_tensor("wkv1", [128, 13 * 128], BF16, kind="ExternalInput")
    wqd = nc.dram_tensor("wq", [128, 9 * 128], BF16, kind="ExternalInput")

    def mark(phase):
        _PHASES.append((phase, int(nc.get_next_instruction_name().split("-")[1])))

    with ExitStack() as ctx:
        tc = ctx.enter_context(tile.TileContext(nc))

        wpool = ctx.enter_context(tc.tile_pool(name="wpool", bufs=1))
        xpool = ctx.enter_context(tc.tile_pool(name="xpool", bufs=1))
        stpool = ctx.enter_context(tc.tile_pool(name="stpool", bufs=3))
        cvpool = ctx.enter_context(tc.tile_pool(name="cvpool", bufs=1))
        tpool = ctx.enter_context(tc.tile_pool(name="tpool", bufs=1))
        obpool = ctx.enter_context(tc.tile_pool(name="obpool", bufs=2))

        psC = ctx.enter_context(tc.tile_pool(name="psC", bufs=4, space="PSUM"))
        psO = ctx.enter_context(tc.tile_pool(name="psO", bufs=4, space="PSUM"))

        def conv_psum():
            return psC.tile([128, 288], F32, name="ps_conv", tag="conv")

        mark('weights')
        # lhsT weight tiles and the identity arrive pre-packed from the host
        # (pure layout transforms of w1/w2/w3 -- no FLOPs): this removes the
        # on-device transpose machinery from the startup critical path.
        identf128 = wpool.tile([128, 128], F32, name="identf128")
        Wkv0 = wpool.tile([128, 13, 128], BF16, name="Wkv0")
        Wkv1 = wpool.tile([128, 13, 128], BF16, name="Wkv1")
        Wq = wpool.tile([128, 9, 128], BF16, name="Wq")
        wkv0f = Wkv0.rearrange("p a b -> p (a b)")
        # single-tap slots (4-12) land first: they gate the startup strips
        nc.sync.dma_start(
            out=wkv0f[:, 512:1664], in_=wkv0d.rearrange("a b -> a b")[:, 512:1664]
        )
        nc.scalar.dma_start(
            out=wkv0f[:, 0:512], in_=wkv0d.rearrange("a b -> a b")[:, 0:512]
        )
        nc.scalar.dma_start(
            out=Wq.rearrange("p a b -> p (a b)"), in_=wqd.rearrange("a b -> a b")
        )
        nc.scalar.dma_start(out=identf128, in_=idn.rearrange("a b -> a b"))
        nc.scalar.dma_start(
            out=Wkv1.rearrange("p a b -> p (a b)"), in_=wkv1d.rearrange("a b -> a b")
        )
        identf64 = identf128[0:64, 0:64]
        identb64 = wpool.tile([64, 64], BF16, name="identb64")
        nc.scalar.copy(out=identb64, in_=identf64)

        # ------------------------------------------------------------------
        # conv output tiles
        # ------------------------------------------------------------------
        # K_f: patch-class-grouped kf, [kf_i0 | kf_i1] in partition halves.
        # K_q: same for qf. T_v: padded vf, [vf_i1 | vf_i0].
        K_f = cvpool.tile([128, AA * L], BF16, name="K_f")
        K_q = cvpool.tile([128, AA * L], BF16, name="K_q")
        T_v = cvpool.tile([128, NPAD], BF16, name="T_v")
        tvv = T_v.rearrange("p (h w) -> p h w", h=HP, w=HP)

        def tv_borders():
            nc.vector.memset(tvv[:, 0:1, :], 0.0)
            nc.vector.memset(tvv[:, HP - 1 : HP, :], 0.0)
            nc.vector.memset(tvv[:, 1 : HP - 1, 0:1], 0.0)
            nc.vector.memset(tvv[:, 1 : HP - 1, HP - 1 : HP], 0.0)

        # ------------------------------------------------------------------
        # input staging
        # ------------------------------------------------------------------
        N1 = 8  # x1 load chunks (12 rows each): fine-grained conv feed
        rows1 = H // N1
        CHB1 = rows1 * W
        CB1 = rows1 * HP
        rows = H // NCHK
        CHB = rows * W  # chunk elements per channel

        def xt_tile(name):
            # T1_0 / T_q / T1_1 / T2_1 rotate through three buffers: T2_1
            # lands in T1_0's slot once kfvf_i0 has consumed it.
            return xpool.tile([128, NPAD], BF16, name=name, tag="xT", bufs=3)

        def t1_alloc(name):
            """[x1_i | x1_i + HP] 2-copy padded tile, borders pre-zeroed."""
            xp = xt_tile(name)
            v = xp.rearrange("p (h w) -> p h w", h=HP, w=HP)
            nc.vector.memset(v[0:64, 0:1, :], 0.0)
            nc.vector.memset(v[0:64, HP - 1 : HP, :], 0.0)
            nc.vector.memset(v[0:64, 1 : HP - 1, 0:1], 0.0)
            nc.vector.memset(v[0:64, 1 : HP - 1, HP - 1 : HP], 0.0)
            nc.vector.memset(v[64:128, 0 : HP - 1, 0:1], 0.0)
            nc.vector.memset(v[64:128, 0 : HP - 1, HP - 1 : HP], 0.0)
            nc.vector.memset(xp[64:128, (HP - 2) * HP : NPAD], 0.0)
            return xp

        def t1_rows(xp, img, a, b, lower_q=None):
            v = xp.rearrange("p (h w) -> p h w", h=HP, w=HP)
            st = stpool.tile([64, CHB1], BF16, name="st", tag="st", bufs=5)
            stc = st[:, 0 : (b - a) * W]
            nc.gpsimd.dma_start(
                out=stc, in_=x1c[img][:, a:b].rearrange("c h w -> c (h w)")
            )
            stv = stc.rearrange("p (h w) -> p h w", h=b - a, w=W)
            nc.vector.tensor_copy(out=v[0:64, 1 + a : 1 + b, 1 : W + 1], in_=stv)
            (lower_q or nc.sync).dma_start(
                out=v[64:128, a:b, 1 : W + 1], in_=stv
            )

        def t1_chunk(xp, img, g):
            t1_rows(xp, img, g * rows1, (g + 1) * rows1)

        def t2_quarter(t2, T1, g):
            """T2 = [x+2HP | x+2HP+1] via same-partition shift copies: the
            upper half shifts T1's upper by 2 rows, the lower half shifts
            T1's lower (x+HP) by HP+1."""
            e = 2352 * (g + 1)
            nc.sync.dma_start(
                out=t2[0:64, 2352 * g : e], in_=T1[0:64, 196 + 2352 * g : 196 + e]
            )
            nc.sync.dma_start(
                out=t2[64:128, 2352 * g : e], in_=T1[64:128, 99 + 2352 * g : 99 + e]
            )
            if g == 3:
                nc.vector.memset(t2[0:64, 9408:NPAD], 0.0)
                nc.vector.memset(t2[64:128, 9408:NPAD], 0.0)

        def build_Tq():
            """[x2_i0 | x2_i1] padded tile, loaded straight from DRAM with
            strided cast-DMAs in row quarters (nothing on the DVE queue)."""
            xp = xt_tile("T_q")
            v = xp.rearrange("p (h w) -> p h w", h=HP, w=HP)
            nc.vector.memset(v[:, 0:1, :], 0.0)
            nc.vector.memset(v[:, HP - 1 : HP, :], 0.0)
            nc.vector.memset(v[:, 1 : HP - 1, 0:1], 0.0)
            nc.vector.memset(v[:, 1 : HP - 1, HP - 1 : HP], 0.0)
            xcv0 = x2c[0].rearrange("c (g h) w -> g c h w", g=4)
            xcv1 = x2c[1].rearrange("c (g h) w -> g c h w", g=4)
            for g in range(4):
                r0 = 1 + g * 24
                nc.gpsimd.dma_start(
                    out=v[0:64, r0 : r0 + 24, 1 : W + 1], in_=xcv0[g]
                )
                nc.gpsimd.dma_start(
                    out=v[64:128, r0 : r0 + 24, 1 : W + 1], in_=xcv1[g]
                )
            return xp

        mark('loads')
        T1_0 = t1_alloc("T1_0")
        # the very first rows land as a half-chunk so strip 0 starts sooner
        t1_rows(T1_0, 0, 0, 6)
        t1_rows(T1_0, 0, 6, 12)
        for g in range(1, N1):
            t1_chunk(T1_0, 0, g)

        # ------------------------------------------------------------------
        # convolutions (3-row grouped strips, N=288)
        # ------------------------------------------------------------------
        # chunk-major grouped layout: chunk s (128 patches) of all 9 classes
        # is contiguous, so one DMA transpose per chunk can fire as soon as
        # its 4 source strips are done (overlapping the conv itself).
        kfA = K_f.rearrange("p (s ki kj l) -> p s ki kj l", s=8, ki=3, kj=3)
        kqA = K_q.rearrange("p (s ki kj l) -> p s ki kj l", s=8, ki=3, kj=3)

        def gr(xv, h0, hs, dj):
            return xv[:, h0 + hs : h0 + hs + 3, dj : dj + W].rearrange(
                "p ki (b kj) -> p ki kj b", kj=3
            )

        def kfvf_strip(img, T1, T2, Wkv, r, use_t2, singles_only=False):
            x1v = T1.rearrange("p (h w) -> p h w", h=HP, w=HP)
            h0 = 3 * r
            ps = conv_psum()
            pv = ps.rearrange("p (a b c) -> p a b c", a=3, b=3)
            grl = lambda hs, dj: gr(x1v[0:64], h0, hs, dj)
            if singles_only:
                # 9 K=64 single-tap matmuls touching only T1's upper half --
                # lets the first strips run before the row-shifted copy lands
                n = 0
                for slot, (hs, dj) in SINGLE_SLOTS:
                    nc.tensor.matmul(
                        pv,
                        Wkv[0:64, slot, :],
                        grl(hs, dj),
                        start=(n == 0),
                        stop=(n == 8),
                    )
                    n += 1
            else:
                for j in range(3):
                    nc.tensor.matmul(
                        pv, Wkv[:, j, :], gr(x1v, h0, 0, j), start=(j == 0), stop=False
                    )
                if use_t2:
                    # column-pair (2,0)&(2,1) via the [x+2HP | x+2HP+1] tile,
                    # then the lone (2,2) single
                    t2v = T2.rearrange("p (h w) -> p h w", h=HP, w=HP)
                    nc.tensor.matmul(
                        pv, Wkv[:, 3, :], gr(t2v, h0, 0, 0), start=False, stop=False
                    )
                    nc.tensor.matmul(
                        pv, Wkv[0:64, 4, :], grl(2, 2), start=False, stop=True
                    )
                else:
                    for n, (slot, dj) in enumerate(((5, 0), (6, 1), (4, 2))):
                        nc.tensor.matmul(
                            pv,
                            Wkv[0:64, slot, :],
                            grl(2, dj),
                            start=False,
                            stop=(n == 2),
                        )
            pk = ps.rearrange("p (ki kj c) -> p ki kj c", ki=3, kj=3)
            kdst = kfA[:, r // 4, :, :, 32 * (r % 4) : 32 * (r % 4) + 32]
            vdst = tvv[:, 1 + h0 : 4 + h0, 1 : W + 1].rearrange(
                "p ki (b kj) -> p ki kj b", kj=3
            )
            if img == 0:
                # psum = [kf_i0 | vf_i0]
                nc.scalar.copy(out=kdst[0:64], in_=pk[0:64])
                nc.vector.tensor_copy(out=vdst[64:128], in_=pk[64:128])
            else:
                # psum = [vf_i1 | kf_i1]
                nc.vector.tensor_copy(out=vdst[0:64], in_=pk[0:64])
                nc.scalar.copy(out=kdst[64:128], in_=pk[64:128])

        def qf_strip(r):
            xqv = T_q.rearrange("p (h w) -> p h w", h=HP, w=HP)
            h0 = 3 * r
            ps = conv_psum()
            pv = ps.rearrange("p (a b c) -> p a b c", a=3, b=3)
            t = 0
            for hs in range(3):
                for dj in range(3):
                    nc.tensor.matmul(
                        pv,
                        Wq[:, t, :],
                        gr(xqv, h0, hs, dj),
                        start=(t == 0),
                        stop=(t == 8),
                    )
                    t += 1
            pk = ps.rearrange("p (ki kj c) -> p ki kj c", ki=3, kj=3)
            qdst = kqA[:, r // 4, :, :, 32 * (r % 4) : 32 * (r % 4) + 32]
            nc.scalar.copy(out=qdst, in_=pk)

        # transposes: [64, (4 chunks)*9*128] -> [128, 4, 9, 64] per image
        # half, batched 2 per image to keep the HWDGE instruction count low
        # (out[p, s, k, c] = chunk s, class k, local patch p)
        kTall = [
            tpool.tile([128, 8, AA, 64], BF16, name=f"kTall{i}") for i in range(2)
        ]
        qTall = [
            tpool.tile([128, 8, AA, 64], BF16, name=f"qTall{i}") for i in range(2)
        ]

        def half_transpose(dst, img, K_tile, h):
            half = slice(0, 64) if img == 0 else slice(64, 128)
            nc.sync.dma_start_transpose(
                dst[img][:, 4 * h : 4 * h + 4],
                K_tile[half, 4608 * h : 4608 * (h + 1)],
            )

        def half_transpose2(dst, img, K_tile, h2):
            half = slice(0, 64) if img == 0 else slice(64, 128)
            nc.sync.dma_start_transpose(
                dst[img][:, 2 * h2 : 2 * h2 + 2],
                K_tile[half, 2304 * h2 : 2304 * (h2 + 1)],
            )

        def half_transpose1(dst, img, K_tile, h1):
            half = slice(0, 64) if img == 0 else slice(64, 128)
            nc.sync.dma_start_transpose(
                dst[img][:, h1 : h1 + 1],
                K_tile[half, 1152 * h1 : 1152 * (h1 + 1)],
            )

        def half_transpose_k(dst, img, K_tile, h1, k0, k1):
            # class-sliced chunk transpose (contiguous 2D input): the final
            # chunk's scores-gating transfer splits so classes k0..k1-1
            # unblock after a half-size transfer
            half = slice(0, 64) if img == 0 else slice(64, 128)
            nc.sync.dma_start_transpose(
                dst[img][:, h1, k0:k1],
                K_tile[half, 1152 * h1 + 128 * k0 : 1152 * h1 + 128 * k1],
            )

        # schedule: kfvf i0; qf in the middle (its 35us window hides image-1
        # input staging and the T2_1 build, and its 1080ns strips leave evac
        # slack); kfvf i1 with the pair-tap T2 last.
        mark('kfvf_i0')
        for r in range(NSTRIP):
            kfvf_strip(0, T1_0, None, Wkv0, r, use_t2=False, singles_only=(r < 1))
            if r % 16 == 15:
                half_transpose(kTall, 0, K_f, r // 16)
        tv_borders()
        T_q = build_Tq()
        T1_1 = t1_alloc("T1_1")
        T2_1 = xt_tile("T2_1")
        _T2_AFTER = {2: 0, 4: 1, 6: 2, 7: 3}
        for g in range(N1):
            t1_chunk(T1_1, 1, g)
            if g in _T2_AFTER:
                t2_quarter(T2_1, T1_1, _T2_AFTER[g])
        mark('qf')
        for r in range(NSTRIP):
            qf_strip(r)
            if r % 16 == 15:
                half_transpose(qTall, 0, K_q, r // 16)
                half_transpose(qTall, 1, K_q, r // 16)
        mark('kfvf_i1')
        for r in range(NSTRIP):
            kfvf_strip(1, T1_1, T2_1, Wkv1, r, use_t2=True)
            if r in (7, 15):
                half_transpose2(kTall, 1, K_f, r // 8)
            elif r in (19, 23, 27):
                half_transpose1(kTall, 1, K_f, (r - 19) // 4 + 4)
            elif r == 31:
                half_transpose_k(kTall, 1, K_f, 7, 0, 4)
                half_transpose_k(kTall, 1, K_f, 7, 4, 9)

        # ------------------------------------------------------------------
        # scores + softmax + attn kernel transposes
        # ------------------------------------------------------------------
        # sc rows = d (partitions 0-63), both images side by side in free dim.
        mark('scores')
        attnP = [
            cvpool.tile([64, AA, 64], BF16, name=f"attnP{i}") for i in range(2)
        ]
        ebias = cvpool.tile([64, 1], F32, name="ebias")
        nc.vector.memset(ebias, EXP_BIAS)
        # attnT[:, t, :]: K rows 0-63 = c of i1 -> M cols 0-63 = d of i1;
        # K rows 64-127 = c of i0 -> M cols 64-127 = d of i0; off-diag zero.
        attnT = cvpool.tile([128, AA, 128], BF16, name="attnT")
        nc.vector.memset(attnT, 0.0)

        def scores_img(img, k0, k1, ps):
            # classes k0..k1-1 accumulate side by side in one PSUM bank and
            # leave through the softmax exp directly (no sc staging tile).
            # class-major: exactly one open accumulation group per bank at a
            # time (interleaved open groups misaccumulate on hardware).
            for k in range(k0, k1):
                po = ps[0:64, 64 * (k - k0) : 64 * (k - k0) + 64]
                for s in range(8):
                    nc.tensor.matmul(
                        po,
                        qTall[img][:, s, k, :],
                        kTall[img][:, s, k, :],
                        start=(s == 0),
                        stop=(s == 7),
                    )

        # attnT holds UNNORMALIZED exp values; the per-row 1/sum factors are
        # gathered into rs128 (rows 0-63 = img1, 64-127 = img0, matching the
        # output einsum psum rows) and applied by the output-copy activation.
        rs128 = cvpool.tile([128, 1], F32, name="rs128")

        sm_acc = [
            [cvpool.tile([64, 1], F32, name=f"sm{i}{j}") for j in range(3)]
            for i in range(2)
        ]
        _EXP_RNG = [slice(0, 256), slice(256, 512), slice(512, 576)]

        def exp_part(img, ps, part):
            """exp straight out of one scores PSUM sub-range; fires as soon
            as those classes' accumulation stops (others may still run)."""
            exf = attnP[img].rearrange("p a c -> p (a c)")
            nc.scalar.activation(
                out=exf[:, _EXP_RNG[part]],
                in_=ps,
                func=mybir.ActivationFunctionType.Exp,
                bias=ebias,
                scale=SCALE,
                accum_out=sm_acc[img][part],
            )

        def finish_softmax(img):
            smp = cvpool.tile([64, 1], F32, name=f"smp{img}")
            nc.scalar.add(out=smp, in_=sm_acc[img][0], add=sm_acc[img][1])
            sm = cvpool.tile([64, 1], F32, name=f"sm{img}")
            nc.scalar.add(out=sm, in_=smp, add=sm_acc[img][2])
            rs = cvpool.tile([64, 1], F32, name=f"rs{img}")
            nc.vector.reciprocal(rs, sm)
            # cross-partition move via DMA; latency is hidden (needed only
            # by the first output copy, ~15us later)
            nc.sync.dma_start(
                out=rs128[64:128, :] if img == 0 else rs128[0:64, :], in_=rs
            )

        # pipeline: img0 scores+softmax+attnT-half run while img1's kT
        # transposes land; only img1's chain sits on the critical path.
        def attn_t_batch(half, t0, nt=3):
            # attnT built in 3-tap batches out of the (now idle) conv psum
            # pool -- 4-deep rotation so batches never wait on each other
            ps = psC.tile([128, 64 * nt], F32, name="ps_t", tag="conv")
            pv = ps.rearrange("p (t c) -> p t c", t=nt)
            for t in range(t0, t0 + nt):
                o = 64 * (t - t0)
                if half == 0:
                    nc.tensor.matmul(
                        ps[64:128, o : o + 64], attnP[0][:, t, :], identb64
                    )
                else:
                    nc.tensor.matmul(
                        ps[0:64, o : o + 64], attnP[1][:, t, :], identb64
                    )
            if half == 0:
                nc.vector.tensor_copy(
                    out=attnT[64:128, t0 : t0 + nt, 64:128],
                    in_=pv[64:128, 0:nt, :],
                )
            else:
                nc.vector.tensor_copy(
                    out=attnT[0:64, t0 : t0 + nt, 0:64],
                    in_=pv[0:64, 0:nt, :],
                )

        psA0 = psO.tile([64, 512], F32, name="ps_sA", tag="po")
        psB0 = psO.tile([64, 64], F32, name="ps_sB", tag="po")
        scores_img(0, 0, 4, psA0[:, 0:256])
        exp_part(0, psA0[:, 0:256], 0)
        scores_img(0, 4, 8, psA0[:, 256:512])
        exp_part(0, psA0[:, 256:512], 1)
        scores_img(0, 8, 9, psB0)
        exp_part(0, psB0, 2)
        finish_softmax(0)
        psA1 = psO.tile([64, 512], F32, name="ps_sA", tag="po")
        psB1 = psO.tile([64, 64], F32, name="ps_sB", tag="po")
        attn_t_batch(0, 0)
        scores_img(1, 0, 4, psA1[:, 0:256])
        exp_part(1, psA1[:, 0:256], 0)
        attn_t_batch(0, 3)
        scores_img(1, 4, 8, psA1[:, 256:512])
        exp_part(1, psA1[:, 256:512], 1)
        attn_t_batch(0, 6)
        scores_img(1, 8, 9, psB1)
        exp_part(1, psB1, 2)
        finish_softmax(1)
        mark('softmax')
        attn_t_batch(1, 0)
        attn_t_batch(1, 3)
        attn_t_batch(1, 6)

        mark('cein')
        y0 = yc[0].rearrange("c h w -> c (h w)")
        y1 = yc[1].rearrange("c h w -> c (h w)")
        def c_strip(h0, nrow, last=False):
            npix = nrow * W
            ps = psO.tile([128, 384], F32, name="ps_o", tag="po")
            pov = ps[:, 0:npix].rearrange("p (a c) -> p a c", a=nrow)
            t = 0
            for ti in range(3):
                for tj in range(3):
                    nc.tensor.matmul(
                        pov,
                        attnT[:, t, :],
                        tvv[:, h0 + ti : h0 + ti + nrow, tj : tj + W],
                        start=(t == 0),
                        stop=(t == 8),
                    )
                    t += 1
            ob = obpool.tile([128, 384], F32, name="outb", tag="outb", bufs=3)
            nc.scalar.activation(
                out=ob[:, 0:npix],
                in_=ps[:, 0:npix],
                func=mybir.ActivationFunctionType.Copy,
                scale=rs128,
            )
            # y1 transfers alternate sync/scalar so neither sequencer's
            # per-DMA hold backlogs the final strip
            q1 = nc.sync if (h0 // 4) % 2 == 0 else nc.scalar
            q1.dma_start(
                out=y1[:, W * h0 : W * (h0 + nrow)], in_=ob[0:64, 0:npix]
            )
            # image-0's transfers ride the idle Pool queue throughout so the
            # sync queue only carries part of the output DMAs
            nc.gpsimd.dma_start(
                out=y0[:, W * h0 : W * (h0 + nrow)], in_=ob[64:128, 0:npix]
            )

        for r in range(VSTRIP):
            c_strip(4 * r, 4, last=(r == VSTRIP - 1))

        if _DEBUG:
            dbg = {
                "dK_f": K_f, "dK_q": K_q, "dT_v": T_v,
                "dkT0": kTall[0].rearrange("p a k c -> p (a k c)"),
                "dqT0": qTall[0].rearrange("p a k c -> p (a k c)"),
                "daP0": attnP[0].rearrange("p a c -> p (a c)"),
                "daP1": attnP[1].rearrange("p a c -> p (a c)"),
                "datT": attnT.rearrange("p a c -> p (a c)"),
                "drs": rs128,
            }
            for nm, t in dbg.items():
                sh = [t.shape[0], int(np.prod(t.shape[1:]))]
                dt_ = F32 if nm == "drs" else (BF16 if t.dtype == BF16 else F32)
                d = nc.dram_tensor(nm, sh, t.dtype, kind="ExternalOutput")
                nc.sync.dma_start(out=d.rearrange("a b -> a b"), in_=t)

    mark('end')
    nc.compile()
    return nc


def _get_program():
    if "nc" not in _CACHE:
        _CACHE["nc"] = _build_program()
    return _CACHE["nc"]


def _pack_kv(wa, wb):
    """lhsT tile for the merged kf|vf conv: [K=128, slot, M=128] flattened."""
    out = np.zeros((128, 13, 128), np.float32)
    for j in range(3):
        for a in range(2):
            out[a * 64 : (a + 1) * 64, j, 0:64] = wa[:, :, a, j].T
            out[a * 64 : (a + 1) * 64, j, 64:128] = wb[:, :, a, j].T
    for b in range(2):
        out[b * 64 : (b + 1) * 64, 3, 0:64] = wa[:, :, 2, b].T
        out[b * 64 : (b + 1) * 64, 3, 64:128] = wb[:, :, 2, b].T
    for slot, dj in ((4, 2), (5, 0), (6, 1)):
        out[0:64, slot, 0:64] = wa[:, :, 2, dj].T
        out[0:64, slot, 64:128] = wb[:, :, 2, dj].T
    slot = 7
    for a in range(2):
        for dj in range(3):
            out[0:64, slot, 0:64] = wa[:, :, a, dj].T
            out[0:64, slot, 64:128] = wb[:, :, a, dj].T
            slot += 1
    import ml_dtypes

    return out.reshape(128, 13 * 128).astype(ml_dtypes.bfloat16)


def _pack_q(w2):
    """Block-diagonal image-merged qf lhsT tiles."""
    out = np.zeros((128, 9, 128), np.float32)
    t = 0
    for ti in range(3):
        for tj in range(3):
            blk = w2[:, :, ti, tj].T
            out[0:64, t, 0:64] = blk
            out[64:128, t, 64:128] = blk
            t += 1
    import ml_dtypes

    return out.reshape(128, 9 * 128).astype(ml_dtypes.bfloat16)


def kernel(x1, x2, w1, w2, w3, **kwargs):
    x1 = np.ascontiguousarray(np.asarray(x1, dtype=np.float32))
    x2 = np.ascontiguousarray(np.asarray(x2, dtype=np.float32))
    w1 = np.ascontiguousarray(np.asarray(w1, dtype=np.float32))
    w2 = np.ascontiguousarray(np.asarray(w2, dtype=np.float32))
    w3 = np.ascontiguousarray(np.asarray(w3, dtype=np.float32))
    wkv0 = _pack_kv(w1, w3)
    wkv1 = _pack_kv(w3, w1)
    wq = _pack_q(w2)

    nc = _get_program()
    in_maps = [
        {
            "x1c": x1[i * BPC : (i + 1) * BPC],
            "x2c": x2[i * BPC : (i + 1) * BPC],
            "w1": w1,
            "w2": w2,
            "w3": w3,
            "idn": _IDN,
            "wkv0": wkv0,
            "wkv1": wkv1,
            "wq": wq,
        }
        for i in range(NCORES)
    ]
    try:
        res = run_bass_kernel_spmd(
            nc, in_maps, core_ids=list(range(NCORES)), **kwargs
        )
    except Exception:
        # one retry: transient device state can fail a first attempt
        res = run_bass_kernel_spmd(
            nc, in_maps, core_ids=list(range(NCORES)), **kwargs
        )
    out = np.concatenate([r["yc"] for r in res.results], axis=0)
    if kwargs:
        return out.astype(np.float32), res
    return out.astype(np.float32)

